# revision 5
# baseline (speedup 1.0000x reference)
"""Contextual loss (cosine distance, 'regular') on 8 Trainium2 cores.

Math (N=1, C=256, S=96*96=9216):
  mean_T = gt.mean over spatial; I/T centered by mean_T, L2-normalized along C.
  cos[i,j] = Iv[:,i] . Tv[:,j]                       (S x S, via matmul over C)
  dist = clip((1-cos)/2, 0); rel = dist/(rowmin+eps); w = exp((1-rel)/0.5)
  cs = w / rowsum(w); loss = -log(mean_j max_i cs[i,j])

Key simplification (verified: clip never triggers for this data since
max cos ~= 0.52): with m_i = max(0,(1-rowmax_i)/2) and s_i = 1/(m_i+eps),
  cs[i,j] = exp(s_i*cos[i,j] - s_i) / Z_i,   Z_i = sum_j exp(s_i*cos[i,j] - s_i)
(the e^2 factor of the reference cancels in the normalization).

Sharding: rows i split 8 ways (1152 rows/core); each core owns its rows'
min/exp/sum completely and emits colmax[128, S] = max over its 9 row-blocks.
Host finishes with max over (8 cores x 128 partitions), mean, -log.

Per 128-row block on device (flash style, nothing S x S ever hits DRAM):
  sweep1: matmul (fp16 in, fp32 PSUM) -> fused PSUM-evac + row-max via
          tensor_tensor_reduce -> cos fp16 in SBUF
  stats:  m = relu(0.5 - 0.5*rowmax) + eps; s = 1/m (DVE reciprocal)
  sweep2: one Exp activation with per-partition scale=s bias=-s,
          fused row-sum via accum_out -> w fp16, Z fp32
  sweep3: fused (w * invZ) max colmax via scalar_tensor_tensor
"""

import numpy as np

C = 256
S = 96 * 96            # 9216
N_CORES = 8
ROWS_PER_CORE = S // N_CORES   # 1152
BLOCKS = ROWS_PER_CORE // 128  # 9
PSUM_FREE = 2048               # psum tile free size (4 banks)
MM_FREE = 512                  # matmul moving free size (1 bank)
EPS_REL = 1e-5

_compiled = None


def _build():
    import concourse.bass as bass
    import concourse.tile as tile
    from concourse import bacc, mybir

    f16 = mybir.dt.float16
    f32 = mybir.dt.float32

    nc = bacc.Bacc("TRN2", target_bir_lowering=False, debug=False,
                   num_devices=N_CORES)
    iv_d = nc.dram_tensor("iv", [C, ROWS_PER_CORE], f16, kind="ExternalInput")
    tv_d = nc.dram_tensor("tv", [C, S], f16, kind="ExternalInput")
    out_d = nc.dram_tensor("colmax", [128, S], f16, kind="ExternalOutput")

    # j-tiles per block: 4 x 2048 + 1 x 1024
    jt_sizes = []
    off = 0
    while off < S:
        sz = min(PSUM_FREE, S - off)
        jt_sizes.append((off, sz))
        off += sz

    HALF = S // 2  # sweep2/3 chunk

    with tile.TileContext(nc) as tc:
        with (
            tc.tile_pool(name="persist", bufs=1) as persist,
            tc.tile_pool(name="blocks", bufs=2) as blk,
            tc.tile_pool(name="stats", bufs=3) as st,
            tc.tile_pool(name="psum", bufs=2, space="PSUM") as pp,
        ):
            iv_sb = [persist.tile([128, ROWS_PER_CORE], f16, tag=f"iv{k}",
                                  name=f"iv_sb{k}") for k in range(2)]
            tv_sb = [persist.tile([128, S], f16, tag=f"tv{k}",
                                  name=f"tv_sb{k}") for k in range(2)]
            for k in range(2):
                nc.sync.dma_start(out=iv_sb[k][:], in_=iv_d[k * 128:(k + 1) * 128, :])
                for (joff, jsz) in jt_sizes:
                    nc.sync.dma_start(out=tv_sb[k][:, joff:joff + jsz],
                                      in_=tv_d[k * 128:(k + 1) * 128, joff:joff + jsz])

            colmax = persist.tile([128, S], f16, tag="colmax")
            nc.gpsimd.memset(colmax[:], 0.0)

            for b in range(BLOCKS):
                bsl = slice(b * 128, (b + 1) * 128)
                cos = blk.tile([128, S], f16, tag="cos")

                for ti, (joff, jsz) in enumerate(jt_sizes):
                    ps = pp.tile([128, PSUM_FREE], f32, tag="ps")
                    for k in range(2):
                        for q in range(jsz // MM_FREE):
                            nc.tensor.matmul(
                                ps[:, q * MM_FREE:(q + 1) * MM_FREE],
                                iv_sb[k][:, bsl],
                                tv_sb[k][:, joff + q * MM_FREE:joff + (q + 1) * MM_FREE],
                                start=(k == 0), stop=(k == 1),
                            )
                    # PSUM evac (fp32 -> fp16), alternating DVE/ScalarE
                    if ti % 2 == 0:
                        nc.vector.tensor_copy(cos[:, joff:joff + jsz], ps[:, :jsz])
                    else:
                        nc.scalar.copy(cos[:, joff:joff + jsz], ps[:, :jsz])

                # row max via fp16 TT-max tree (2x DVE mode) + final reduce
                racc = st.tile([128, 1024], f16, tag="racc")
                nc.vector.tensor_copy(racc[:], cos[:, 0:1024])
                for r in range(1, S // 1024):
                    nc.vector.tensor_max(racc[:], racc[:],
                                         cos[:, r * 1024:(r + 1) * 1024])
                rowmax = st.tile([128, 1], f32, tag="rowmax")
                nc.vector.reduce_max(rowmax[:], racc[:], axis=mybir.AxisListType.X)
                # me = max(0,(1-rowmax)/2) + eps, as (-0.5*rowmax + 0.5+eps)
                # clamped below at eps (clamp only matters if rowmax > 1).
                me = st.tile([128, 1], f32, tag="me")
                nc.vector.tensor_scalar(me[:], rowmax[:], -0.5, 0.5 + EPS_REL,
                                        op0=mybir.AluOpType.mult,
                                        op1=mybir.AluOpType.add)
                nc.vector.tensor_scalar_max(me[:], me[:], EPS_REL)
                s_t = st.tile([128, 1], f32, tag="s")
                nc.vector.reciprocal(s_t[:], me[:])
                neg_s = st.tile([128, 1], f32, tag="neg_s")
                nc.vector.tensor_scalar_mul(neg_s[:], s_t[:], -1.0)

                w = blk.tile([128, S], f16, tag="w")
                zpart = st.tile([128, 2], f32, tag="zpart")
                for h in range(2):
                    hsl = slice(h * HALF, (h + 1) * HALF)
                    nc.scalar.activation(w[:, hsl], cos[:, hsl],
                                         mybir.ActivationFunctionType.Exp,
                                         bias=neg_s[:], scale=s_t[:],
                                         accum_out=zpart[:, h:h + 1])

                z = st.tile([128, 1], f32, tag="z")
                nc.vector.reduce_sum(z[:], zpart[:], axis=mybir.AxisListType.X)
                inv_z = st.tile([128, 1], f32, tag="inv_z")
                nc.vector.reciprocal(inv_z[:], z[:])

                for h in range(2):
                    hsl = slice(h * HALF, (h + 1) * HALF)
                    nc.vector.scalar_tensor_tensor(
                        out=colmax[:, hsl],
                        in0=w[:, hsl],
                        scalar=inv_z[:],
                        in1=colmax[:, hsl],
                        op0=mybir.AluOpType.mult,
                        op1=mybir.AluOpType.max,
                    )

            nc.sync.dma_start(out=out_d[:], in_=colmax[:])

    nc.compile()
    return nc


def _get_compiled():
    global _compiled
    if _compiled is None:
        _compiled = _build()
    return _compiled


def _preprocess(images: np.ndarray, gt: np.ndarray):
    x = np.asarray(images, np.float32)[0].reshape(C, S)
    t = np.asarray(gt, np.float32)[0].reshape(C, S)
    mean_t = t.mean(axis=1, dtype=np.float32).astype(np.float32)
    i_c = x - mean_t[:, None]
    t_c = t - mean_t[:, None]
    i_n = np.sqrt((i_c * i_c).sum(axis=0, dtype=np.float32)).astype(np.float32)
    t_n = np.sqrt((t_c * t_c).sum(axis=0, dtype=np.float32)).astype(np.float32)
    iv = (i_c / np.maximum(i_n, 1e-12)).astype(np.float16)
    tv = (t_c / np.maximum(t_n, 1e-12)).astype(np.float16)
    return iv, tv


def kernel(images: np.ndarray, gt: np.ndarray) -> np.ndarray:
    from concourse.bass_utils import run_bass_kernel_spmd

    nc = _get_compiled()
    iv, tv = _preprocess(images, gt)
    in_maps = [
        {"iv": np.ascontiguousarray(iv[:, c * ROWS_PER_CORE:(c + 1) * ROWS_PER_CORE]),
         "tv": tv}
        for c in range(N_CORES)
    ]
    res = run_bass_kernel_spmd(nc, in_maps, list(range(N_CORES)))
    colmax = np.stack([res.results[c]["colmax"] for c in range(N_CORES)])
    cs_max = colmax.astype(np.float32).max(axis=(0, 1))       # [S]
    loss = -np.log(cs_max.mean(dtype=np.float32))
    return np.asarray(loss, dtype=np.float32)
